# revision 27
# baseline (speedup 1.0000x reference)
"""Griffin recurrence Trainium2 kernel, v3 (fp8 DoubleRow matmul).

Sharding: 8 cores = 4 batches x 2 channel-halves (192 channels each).
Matmul runs in fp8e4 with MatmulPerfMode.DoubleRow (256-deep contraction
per instruction, 0.5 cycles per output column = 4x the bf16 rate) using a
3-pass hi/lo split that keeps every pass at one common PSUM scale:

    xhi32 = 32*fp8(x)            (exact exponent shift)
    xlo32 = fp8(32*(x - fp8(x)))
    Whi   = fp8(64*W)
    Wlo   = fp8(64*W - Whi)      (direct, subnormals fine)
    psum  = xhi32@Whi + xlo32@Whi + xhi32@Wlo = 2048*(x@W) + O(err)

The 1/2048 descale is folded into the ACT scale arguments and into the
fitted q-sigmoid output coefficients, so no extra elementwise ops.
The chunked scan is identical to v2: per-chunk cumulative decay via a
multiplicative tensor_tensor_scan, clipped division u*recip(max(D,1e-10)),
cumsum scan, and an incremental cross-chunk scan chained through AP scan
initials; sqrt(1-a^2) is a fitted sum of sigmoids so the ACT engine never
swaps activation tables. Elementwise work is split across Pool (gpsimd)
and DVE; output is written bf16.

HW-legality notes (birverifier/codegen): GPSIMD cannot access PSUM;
scalar_tensor_tensor is DVE-only; TT divide is not a valid HW ALU op.
"""

import sys

sys.path.insert(0, "/opt/trn_rl_repo")

from contextlib import ExitStack

import numpy as np
import ml_dtypes

from concourse import bacc, mybir, tile
from concourse.bass_utils import run_bass_kernel_spmd

f32 = mybir.dt.float32
bf16 = mybir.dt.bfloat16
f8 = mybir.dt.float8e4
AF = mybir.ActivationFunctionType
ALU = mybir.AluOpType
PM = mybir.MatmulPerfMode

D_MODEL = 2048
D_REC = 384
CHUNK = 64
NCORE = 8
CH = 192           # channels per core
SCW = 512          # seq-tile width in the steady state
NK = D_MODEL // 128  # 16 k-subtiles (8 DoubleRow supertiles)
NKS = NK // 2      # 8 k-supertiles of 256
NT = 5             # M-tiles (640 = 5*128 packed W rows)
EPS_LOG = 1e-10
PS_SCALE = 1.0 / 2048.0  # psum holds 2048*z

E4 = ml_dtypes.float8_e4m3

# q(p) = sqrt(1 - sigmoid(p)^2) ~= C0 + sum_i Ci*sigmoid(Ai*p + Bi),
# max abs err 8.8e-4 over p in [-14, 14]. All terms live in the sigmoid
# ACT table, so the kernel never swaps activation tables. The output
# coefficients are multiplied by PS_SCALE at emit time to fold in the
# fp8 psum descale (q' = q/2048 so u = q'*(i*psv) is true-scale).
QC0 = 0.697865
QTERMS = (
    (0.302546, -0.537589, 1.749442),
    (-3.0, 0.831839, -0.394654),
    (2.303064, 0.830397, -0.058968),
)

_built = {}


def _emit(tc, nc, xhiT, xloT, whiT, wloT, db0, db1, out, seq):
    nsc = seq // SCW
    nch = seq // CHUNK        # 64 chunks total

    with ExitStack() as ctx:
        const = ctx.enter_context(tc.tile_pool(name="const", bufs=1))
        sm = ctx.enter_context(tc.tile_pool(name="sm", bufs=1))
        xp = ctx.enter_context(tc.tile_pool(name="xp", bufs=3))
        pp = ctx.enter_context(tc.tile_pool(name="pp", bufs=1, space="PSUM"))
        pv = ctx.enter_context(tc.tile_pool(name="pv", bufs=2, space="PSUM"))
        pv0 = ctx.enter_context(tc.tile_pool(name="pv0", bufs=2, space="PSUM"))
        wk = ctx.enter_context(tc.tile_pool(name="wk", bufs=2))

        # constants. x stream owns the SP queue; W/db go on the ACT queue
        # (k-ascending W so the PE can start immediately).
        zeros = const.tile([128, CHUNK], f32, tag="zeros")
        nc.vector.memset(zeros[:], 0.0)
        ones = const.tile([128, CHUNK], f32, tag="ones")
        nc.vector.memset(ones[:], 1.0)
        # W-scan mask: ones with zeros at chunk starts (segmented cumsum)
        mask = const.tile([128, SCW], f32, tag="mask")
        nc.vector.memset(mask[:], 1.0)
        for c0_ in range(0, SCW, CHUNK):
            nc.vector.memset(mask[:, c0_ : c0_ + 1], 0.0)
        whi = const.tile([128, NK, NT * 128], f8, tag="whi")
        wlo = const.tile([128, NK, NT * 128], f8, tag="wlo")
        # t3 column of k0 first, on the SP queue: the ACT queue opens with
        # a hoisted LoadActFuncSet (1.3us) that would gate the first
        # matmul's weights. The rest of the W/x startup stream is
        # interleaved across the SP and ACT queues (see the s0 == 0 branch
        # below); wlo arrives as one bulk DMA once the hi stream is in.
        nc.sync.dma_start(
            whi[:, 0:2, 384:512],
            whiT[0:256, 384:512].rearrange("(k p) m -> p k m", p=128),
        )
        db0_t = const.tile([128, 1], f32, tag="db0")
        nc.scalar.dma_start(db0_t[:], db0[:])
        db1_t = const.tile([128, 1], f32, tag="db1")
        nc.scalar.dma_start(db1_t[:], db1[:])
        # bias columns for the q sigmoid terms: Ai*db + Bi
        qbA, qbB = [], []
        for idx, (_c, al, be) in enumerate(QTERMS):
            ta = const.tile([128, 1], f32, tag=f"qbA{idx}")
            nc.vector.tensor_scalar(ta[:], db0_t[:], al, be, ALU.mult, ALU.add)
            qbA.append(ta)
            tb = const.tile([64, 1], f32, tag=f"qbB{idx}")
            nc.vector.tensor_scalar(tb[:], db1_t[0:64, :], al, be, ALU.mult, ALU.add)
            qbB.append(tb)

        # cross-chunk chains (persistent, written cpc columns per seq-tile)
        CDa = sm.tile([128, nch], f32, tag="CDa")
        CWa = sm.tile([128, nch], f32, tag="CWa")
        INa = sm.tile([128, nch], f32, tag="INa")
        CDb = sm.tile([64, nch], f32, tag="CDb")
        CWb = sm.tile([64, nch], f32, tag="CWb")
        INb = sm.tile([64, nch], f32, tag="INb")

        # steady-state 512-wide tiles; the last 512 is split (256, 128, 128)
        # so the post-matmul drain chain after the final matmul is short.
        last0 = (nsc - 1) * SCW
        tiles = [(i * SCW, SCW) for i in range(nsc - 1)]
        tiles += [(last0, 256), (last0 + 256, 128), (last0 + 384, 128)]

        for s0, scw in tiles:
            is_last = s0 + scw == seq
            ew = nc.gpsimd
            cpc = scw // CHUNK
            gc0 = s0 // CHUNK

            # both fp8 x streams for this seq-tile; the first tile is split
            # per-k so the PE can start after the first k-slice lands
            xh_t = xp.tile([128, NK, SCW], f8, tag="xh")
            xl_t = xp.tile([128, NK, SCW], f8, tag="xl")
            xh = xh_t[:, :, 0:scw]
            xl = xl_t[:, :, 0:scw]
            if s0 == 0:
                # startup: 2-k pieces first so pass-1 can begin ~3us in, then
                # quarters, split across the SP and ACT queues in the k-major
                # consumption order of the first tile's matmuls
                nc.sync.dma_start(
                    xh[:, 0:2, :],
                    xhiT[0:256, s0 : s0 + scw].rearrange("(k p) s -> p k s", p=128),
                )
                nc.sync.dma_start(
                    whi[:, 0:2, 512:640],
                    whiT[0:256, 512:640].rearrange("(k p) m -> p k m", p=128),
                )
                nc.sync.dma_start(
                    whi[:, 0:2, 0:384],
                    whiT[0:256, 0:384].rearrange("(k p) m -> p k m", p=128),
                )
                for k0_, k1_ in ((2, 4), (4, 8), (8, 12), (12, 16)):
                    nc.sync.dma_start(
                        xh[:, k0_:k1_, :],
                        xhiT[k0_ * 128 : k1_ * 128, s0 : s0 + scw].rearrange(
                            "(k p) s -> p k s", p=128
                        ),
                    )
                for k0_, k1_ in ((2, 4), (4, 8), (8, 12), (12, 16)):
                    nc.scalar.dma_start(
                        whi[:, k0_:k1_, :],
                        whiT[k0_ * 128 : k1_ * 128, :].rearrange(
                            "(k p) m -> p k m", p=128
                        ),
                    )
                for k0_ in range(0, NK, 4):
                    nc.scalar.dma_start(
                        xl[:, k0_ : k0_ + 4, :],
                        xloT[k0_ * 128 : (k0_ + 4) * 128, s0 : s0 + scw].rearrange(
                            "(k p) s -> p k s", p=128
                        ),
                    )
                nc.sync.dma_start(
                    wlo[:, :, :], wloT[:, :].rearrange("(k p) m -> p k m", p=128)
                )
            else:
                nc.sync.dma_start(
                    xh,
                    xhiT[:, s0 : s0 + scw].rearrange("(k p) s -> p k s", p=128),
                )
                nc.sync.dma_start(
                    xl,
                    xloT[:, s0 : s0 + scw].rearrange("(k p) s -> p k s", p=128),
                )
            ps = {}
            nh = (scw + 255) // 256  # N-halves of <=256 cols
            # B-tiles (t3/t4) first so the B drain overlaps the A matmuls.
            # On the final piece: t3 first (so the iB realign DMA's ~2.2us
            # latency hides under the matmuls), t4 (ivB's input) last.
            pool_of = {3: pp, 4: pv, 0: pv0, 1: pp, 2: pv}
            tseq = (3, 0, 1, 2, 4) if is_last else (3, 4, 0, 1, 2)
            torder = tuple((t, pool_of[t]) for t in tseq)
            if s0 == 0:
                # k-major emission for the first tile: every matmul consumes
                # the most recently landed DMA slice, so the PE ramps with
                # the startup stream instead of stalling on pass-2/3 inputs.
                # The two 256-col halves run as separate phases because one
                # psum tile cannot hold two pending accumulation groups.
                for t, pool in torder:
                    p = pool.tile([128, SCW], f32, tag=f"ps{t}")
                    ps[t] = p
                passes = [(xh, whi), (xl, whi), (xh, wlo)]
                for hh in range(nh):
                    n0 = hh * 256
                    n1 = min(n0 + 256, scw)
                    for pi, (xop, wop) in enumerate(passes):
                        for ks in range(NKS):
                            for t, _pool in torder:
                                m0 = t * 128
                                nc.tensor.matmul(
                                    ps[t][:, n0:n1],
                                    wop[:, 2 * ks : 2 * ks + 2, m0 : m0 + 128],
                                    xop[:, 2 * ks : 2 * ks + 2, n0:n1],
                                    start=(pi == 0 and ks == 0),
                                    stop=(pi == 2 and ks == NKS - 1),
                                    perf_mode=PM.DoubleRow,
                                )
                for t, _pool in torder:
                    ps[t] = ps[t][:, 0:scw]
            else:
                for t, pool in torder:
                    p = pool.tile([128, SCW], f32, tag=f"ps{t}")
                    m0 = t * 128
                    for hh in range(nh):
                        n0 = hh * 256
                        n1 = min(n0 + 256, scw)
                        seqs = []
                        for xop, wop in ((xh, whi), (xl, whi), (xh, wlo)):
                            for ks in range(NKS):
                                seqs.append((xop, wop, ks))
                        for j, (xop, wop, ks) in enumerate(seqs):
                            nc.tensor.matmul(
                                p[:, n0:n1],
                                wop[:, 2 * ks : 2 * ks + 2, m0 : m0 + 128],
                                xop[:, 2 * ks : 2 * ks + 2, n0:n1],
                                start=(j == 0),
                                stop=(j == len(seqs) - 1),
                                perf_mode=PM.DoubleRow,
                            )
                    ps[t] = p[:, 0:scw]

            def wkt(tag, pg, dt=f32, width=None):
                if width is not None:  # small chain tiles
                    t_ = wk.tile([pg, 8], dt, tag=tag)
                    return t_[:, 0:width]
                t_ = wk.tile([pg, SCW], dt, tag=tag)
                return t_[:, 0:scw]

            # B-group first throughout: its tiles (t3, t4) finish matmul
            # first, so its whole drain chain overlaps the A matmuls and the
            # end-of-kernel tail is only A's short chain. On the final piece
            # the whole ordering flips to A-major (A tiles drain first, B is
            # the short tail).
            qA = wkt("qA", 128)
            qB = wkt("qB", 64)
            prep_out = {}

            def emit_prep_B():
                ab = wkt("ab", 128)  # [aB; iB] + [db1; 0]
                nc.scalar.activation(
                    ab, ps[3], AF.Sigmoid, bias=db1_t[:], scale=PS_SCALE
                )
                iB = wkt("iB", 64)
                nc.scalar.dma_start(iB, ab[64:128, :])  # realign iB to rows 0..63
                prep_out["B"] = (ab[0:64, :], iB, ps[4][0:64, :])

            def emit_prep_A():
                aA = wkt("aA", 128)
                nc.scalar.activation(
                    aA, ps[0], AF.Sigmoid, bias=db0_t[:], scale=PS_SCALE
                )
                iA = wkt("iA", 128)
                nc.scalar.activation(iA, ps[1], AF.Sigmoid, scale=PS_SCALE)
                prep_out["A"] = (aA, iA, ps[2])

            def emit_q(name):
                pg, src, qb, qt = {
                    "B": (64, ps[3], qbB, qB),
                    "A": (128, ps[0], qbA, qA),
                }[name]
                sgs = []
                for idx, (_c, al, _b) in enumerate(QTERMS):
                    s = wkt(f"qs{name}{idx}", pg)
                    nc.scalar.activation(
                        s, src[0:pg, :], AF.Sigmoid,
                        bias=qb[idx][0:pg, :], scale=al * PS_SCALE,
                    )
                    sgs.append(s)
                # scalar_tensor_tensor is DVE-only on HW; build q on Pool
                # with in-place tensor_scalar + tensor_tensor ops.
                # Output coefficients carry PS_SCALE so q' = q/2048 and the
                # scan input w = q'*R*(i*psv) is true-scale.
                ew.tensor_scalar(
                    sgs[0], sgs[0], QTERMS[0][0] * PS_SCALE, QC0 * PS_SCALE,
                    ALU.mult, ALU.add,
                )
                ew.tensor_scalar(
                    sgs[1], sgs[1], QTERMS[1][0] * PS_SCALE, None, ALU.mult
                )
                ew.tensor_scalar(
                    sgs[2], sgs[2], QTERMS[2][0] * PS_SCALE, None, ALU.mult
                )
                ew.tensor_add(sgs[0], sgs[0], sgs[1])
                ew.tensor_add(qt, sgs[0], sgs[2])

            if is_last:
                emit_prep_B()
                emit_q("B")
                emit_prep_A()
                emit_q("A")
            else:
                emit_prep_B()
                emit_q("B")
                emit_q("A")
                emit_prep_A()

            gparams = {
                "B": ("B", 64, qB, CDb, CWb, INb),
                "A": ("A", 128, qA, CDa, CWa, INa),
            }
            gorder = ("A", "B") if is_last else ("B", "A")
            for gname in gorder:
                name, pg, q_t, CD, CW, IN = gparams[gname]
                a_ap, i_t, v_ps = prep_out[gname]
                # intra-chunk: D = cumprod(a) with chunk resets via
                # one masked scan: D = (a*m)*D_prev + a*(1-m)
                am = wkt(f"am{name}", pg)
                ew.tensor_mul(am, a_ap, mask[0:pg, 0:scw])
                az = wkt(f"az{name}", pg)
                ew.tensor_tensor(az, a_ap, am, ALU.subtract)
                D = wkt(f"D{name}", pg)
                nc.vector.tensor_tensor_scan(
                    D, am, az, 1.0, ALU.mult, ALU.add
                )
                Dc = wkt(f"Dc{name}", pg)
                ew.tensor_scalar_max(Dc, D, EPS_LOG)
                R = wkt(f"R{name}", pg)
                nc.vector.reciprocal_approx_fast(R, Dc)
                # r2 = q*R is ready before iv (iv needs the last matmul in
                # the drain tile), so w = r2*iv is one op after iv lands
                ew.tensor_mul(R, q_t, R)
                iv_t = wkt(f"iv{name}", pg)
                nc.vector.tensor_mul(iv_t, i_t, v_ps)
                w_ = iv_t
                ew.tensor_mul(w_, R, iv_t)
                W = wkt(f"W{name}", pg)
                nc.vector.tensor_tensor_scan(
                    W, mask[0:pg, 0:scw], w_, 0.0, ALU.mult, ALU.add
                )

                # incremental cross-chunk scan on this tile's boundaries
                g = slice(gc0, gc0 + cpc)
                bd = D[:, CHUNK - 1 :: CHUNK]
                bW = W[:, CHUNK - 1 :: CHUNK]
                bdc = wkt(f"bdc{name}", pg, width=cpc)
                ew.tensor_scalar_max(bdc, bd, EPS_LOG)
                itb = wkt(f"itb{name}", pg, width=cpc)
                ew.tensor_mul(itb, bd, bW)
                cd_init = 1.0 if s0 == 0 else CD[:, gc0 - 1 : gc0]
                nc.vector.tensor_tensor_scan(
                    CD[:, g], bdc, zeros[0:pg, 0:cpc], cd_init,
                    ALU.mult, ALU.add,
                )
                CDc = wkt(f"CDc{name}", pg, width=cpc)
                nc.vector.tensor_scalar_max(CDc, CD[:, g], EPS_LOG)
                CDr = wkt(f"CDr{name}", pg, width=cpc)
                nc.vector.reciprocal_approx_fast(CDr, CDc)
                tms = wkt(f"tms{name}", pg, width=cpc)
                nc.vector.tensor_mul(tms, itb, CDr)
                cw_init = 0.0 if s0 == 0 else CW[:, gc0 - 1 : gc0]
                nc.vector.tensor_tensor_scan(
                    CW[:, g], ones[0:pg, 0:cpc], tms, cw_init,
                    ALU.mult, ALU.add,
                )
                nc.vector.tensor_mul(IN[:, g], CD[:, g], CW[:, g])

                # combine: state = (W + inc) * D, written bf16
                ob = wkt(f"ob{name}", pg, dt=bf16)
                for c in range(cpc):
                    gc = gc0 + c
                    cs = slice(c * CHUNK, (c + 1) * CHUNK)
                    inc = (
                        zeros[0:pg, 0:1] if gc == 0
                        else IN[:, gc - 1 : gc]
                    )
                    nc.vector.scalar_tensor_tensor(
                        ob[:, cs], W[:, cs], inc, D[:, cs],
                        ALU.add, ALU.mult,
                    )
                orow = 0 if name == "A" else 128
                nc.scalar.dma_start(
                    out[orow : orow + pg, s0 : s0 + scw], ob
                )


def _build(seq):
    if seq in _built:
        return _built[seq]
    nc = bacc.Bacc(
        "TRN2", target_bir_lowering=False, debug=False, num_devices=NCORE
    )
    xhiT = nc.dram_tensor("xhiT", [D_MODEL, seq], f8, kind="ExternalInput").ap()
    xloT = nc.dram_tensor("xloT", [D_MODEL, seq], f8, kind="ExternalInput").ap()
    whiT = nc.dram_tensor("whiT", [D_MODEL, NT * 128], f8, kind="ExternalInput").ap()
    wloT = nc.dram_tensor("wloT", [D_MODEL, NT * 128], f8, kind="ExternalInput").ap()
    db0 = nc.dram_tensor("db0", [128, 1], f32, kind="ExternalInput").ap()
    db1 = nc.dram_tensor("db1", [128, 1], f32, kind="ExternalInput").ap()
    out = nc.dram_tensor("out", [CH, seq], bf16, kind="ExternalOutput").ap()
    with tile.TileContext(nc) as tc:
        _emit(tc, nc, xhiT, xloT, whiT, wloT, db0, db1, out, seq)
    nc.compile()
    _built[seq] = nc
    return nc


def _pack_w(W, h):
    """Pack this half's W rows into 640 rows of 5 M-tiles.

    t0 = a[0:128], t1 = i[0:128], t2 = v[0:128],
    t3 = [a[128:192]; i[128:192]], t4 = [v[128:192]; zeros]."""
    c0 = h * CH
    z = np.zeros((64, W.shape[1]), np.float32)
    return np.concatenate(
        [
            W[c0 : c0 + 128],
            W[D_REC + c0 : D_REC + c0 + 128],
            W[2 * D_REC + c0 : 2 * D_REC + c0 + 128],
            W[c0 + 128 : c0 + 192],
            W[D_REC + c0 + 128 : D_REC + c0 + 192],
            W[2 * D_REC + c0 + 128 : 2 * D_REC + c0 + 192],
            z,
        ],
        axis=0,
    )


def _in_maps(x, W, db):
    maps = []
    xhi_c, xlo_c = {}, {}
    for core in range(NCORE):
        b, hh = core // 2, core % 2
        if b not in xhi_c:
            xb = x[b]  # [seq, D_MODEL] f32
            xhi = xb.astype(E4).astype(np.float32)
            xhi_c[b] = np.ascontiguousarray((32.0 * xhi).T).astype(E4)
            xlo_c[b] = np.ascontiguousarray((32.0 * (xb - xhi)).T).astype(E4)
        w64 = 64.0 * _pack_w(W, hh)
        whi = w64.astype(E4).astype(np.float32)
        wlo = w64 - whi
        c0 = hh * CH
        db0v = np.ascontiguousarray(db[c0 : c0 + 128].reshape(128, 1))
        db1v = np.ascontiguousarray(
            np.concatenate([db[c0 + 128 : c0 + 192], np.zeros(64, np.float32)]).reshape(
                128, 1
            )
        )
        maps.append(
            {
                "xhiT": xhi_c[b],
                "xloT": xlo_c[b],
                "whiT": np.ascontiguousarray(whi.T).astype(E4),
                "wloT": np.ascontiguousarray(wlo.T).astype(E4),
                "db0": db0v,
                "db1": db1v,
            }
        )
    return maps


def kernel(x, W, decay_bias, _trace=False):
    x = np.asarray(x, np.float32)
    W = np.asarray(W, np.float32)
    db = np.asarray(decay_bias, np.float32)
    B, S, _ = x.shape
    nc = _build(S)
    res = run_bass_kernel_spmd(nc, _in_maps(x, W, db), list(range(NCORE)), trace=_trace)
    outf = np.empty((B, S, D_REC), np.float32)
    for core in range(NCORE):
        b, hh = core // 2, core % 2
        outf[b, :, hh * CH : (hh + 1) * CH] = (
            np.asarray(res.results[core]["out"]).astype(np.float32).T
        )
    if _trace:
        return outf, res
    return outf


# revision 28
# speedup vs baseline: 1.0075x; 1.0075x over previous
"""Griffin recurrence Trainium2 kernel, v3 (fp8 DoubleRow matmul).

Sharding: 8 cores = 4 batches x 2 channel-halves (192 channels each).
Matmul runs in fp8e4 with MatmulPerfMode.DoubleRow (256-deep contraction
per instruction, 0.5 cycles per output column = 4x the bf16 rate) using a
3-pass hi/lo split that keeps every pass at one common PSUM scale:

    xhi32 = 32*fp8(x)            (exact exponent shift)
    xlo32 = fp8(32*(x - fp8(x)))
    Whi   = fp8(64*W)
    Wlo   = fp8(64*W - Whi)      (direct, subnormals fine)
    psum  = xhi32@Whi + xlo32@Whi + xhi32@Wlo = 2048*(x@W) + O(err)

The 1/2048 descale is folded into the ACT scale arguments and into the
fitted q-sigmoid output coefficients, so no extra elementwise ops.
The chunked scan is identical to v2: per-chunk cumulative decay via a
multiplicative tensor_tensor_scan, clipped division u*recip(max(D,1e-10)),
cumsum scan, and an incremental cross-chunk scan chained through AP scan
initials; sqrt(1-a^2) is a fitted sum of sigmoids so the ACT engine never
swaps activation tables. Elementwise work is split across Pool (gpsimd)
and DVE; output is written bf16.

HW-legality notes (birverifier/codegen): GPSIMD cannot access PSUM;
scalar_tensor_tensor is DVE-only; TT divide is not a valid HW ALU op.
"""

import sys

sys.path.insert(0, "/opt/trn_rl_repo")

from contextlib import ExitStack

import numpy as np
import ml_dtypes

from concourse import bacc, mybir, tile
from concourse.bass_utils import run_bass_kernel_spmd

f32 = mybir.dt.float32
bf16 = mybir.dt.bfloat16
f8 = mybir.dt.float8e4
AF = mybir.ActivationFunctionType
ALU = mybir.AluOpType
PM = mybir.MatmulPerfMode

D_MODEL = 2048
D_REC = 384
CHUNK = 64
NCORE = 8
CH = 192           # channels per core
SCW = 512          # seq-tile width in the steady state
NK = D_MODEL // 128  # 16 k-subtiles (8 DoubleRow supertiles)
NKS = NK // 2      # 8 k-supertiles of 256
NT = 5             # M-tiles (640 = 5*128 packed W rows)
EPS_LOG = 1e-10
PS_SCALE = 1.0 / 2048.0  # psum holds 2048*z

E4 = ml_dtypes.float8_e4m3

# q(p) = sqrt(1 - sigmoid(p)^2) ~= C0 + sum_i Ci*sigmoid(Ai*p + Bi),
# max abs err 8.8e-4 over p in [-14, 14]. All terms live in the sigmoid
# ACT table, so the kernel never swaps activation tables. The output
# coefficients are multiplied by PS_SCALE at emit time to fold in the
# fp8 psum descale (q' = q/2048 so u = q'*(i*psv) is true-scale).
QC0 = 0.697865
QTERMS = (
    (0.302546, -0.537589, 1.749442),
    (-3.0, 0.831839, -0.394654),
    (2.303064, 0.830397, -0.058968),
)

_built = {}


def _emit(tc, nc, xhiT, xloT, whiT, wloT, db0, db1, out, seq):
    nsc = seq // SCW
    nch = seq // CHUNK        # 64 chunks total

    with ExitStack() as ctx:
        const = ctx.enter_context(tc.tile_pool(name="const", bufs=1))
        sm = ctx.enter_context(tc.tile_pool(name="sm", bufs=1))
        xp = ctx.enter_context(tc.tile_pool(name="xp", bufs=3))
        pp = ctx.enter_context(tc.tile_pool(name="pp", bufs=1, space="PSUM"))
        pv = ctx.enter_context(tc.tile_pool(name="pv", bufs=2, space="PSUM"))
        pv0 = ctx.enter_context(tc.tile_pool(name="pv0", bufs=2, space="PSUM"))
        wk = ctx.enter_context(tc.tile_pool(name="wk", bufs=2))

        # constants. x stream owns the SP queue; W/db go on the ACT queue
        # (k-ascending W so the PE can start immediately).
        zeros = const.tile([128, CHUNK], f32, tag="zeros")
        nc.vector.memset(zeros[:], 0.0)
        ones = const.tile([128, CHUNK], f32, tag="ones")
        nc.vector.memset(ones[:], 1.0)
        # W-scan mask: ones with zeros at chunk starts (segmented cumsum)
        mask = const.tile([128, SCW], f32, tag="mask")
        nc.vector.memset(mask[:], 1.0)
        for c0_ in range(0, SCW, CHUNK):
            nc.vector.memset(mask[:, c0_ : c0_ + 1], 0.0)
        whi = const.tile([128, NK, NT * 128], f8, tag="whi")
        wlo = const.tile([128, NK, NT * 128], f8, tag="wlo")
        # t3 column of k0 first, on the SP queue: the ACT queue opens with
        # a hoisted LoadActFuncSet (1.3us) that would gate the first
        # matmul's weights. The rest of the W/x startup stream is
        # interleaved across the SP and ACT queues (see the s0 == 0 branch
        # below); wlo arrives as one bulk DMA once the hi stream is in.
        nc.sync.dma_start(
            whi[:, 0:2, 384:512],
            whiT[0:256, 384:512].rearrange("(k p) m -> p k m", p=128),
        )
        db0_t = const.tile([128, 1], f32, tag="db0")
        nc.scalar.dma_start(db0_t[:], db0[:])
        db1_t = const.tile([128, 1], f32, tag="db1")
        nc.scalar.dma_start(db1_t[:], db1[:])
        # bias columns for the q sigmoid terms: Ai*db + Bi
        qbA, qbB = [], []
        for idx, (_c, al, be) in enumerate(QTERMS):
            ta = const.tile([128, 1], f32, tag=f"qbA{idx}")
            nc.vector.tensor_scalar(ta[:], db0_t[:], al, be, ALU.mult, ALU.add)
            qbA.append(ta)
            tb = const.tile([64, 1], f32, tag=f"qbB{idx}")
            nc.vector.tensor_scalar(tb[:], db1_t[0:64, :], al, be, ALU.mult, ALU.add)
            qbB.append(tb)

        # cross-chunk chains (persistent, written cpc columns per seq-tile)
        CDa = sm.tile([128, nch], f32, tag="CDa")
        CWa = sm.tile([128, nch], f32, tag="CWa")
        INa = sm.tile([128, nch], f32, tag="INa")
        CDb = sm.tile([64, nch], f32, tag="CDb")
        CWb = sm.tile([64, nch], f32, tag="CWb")
        INb = sm.tile([64, nch], f32, tag="INb")

        # steady-state 512-wide tiles; the last 512 is split into two 256s so
        # the post-matmul drain chain after the final matmul is short.
        last0 = (nsc - 1) * SCW
        tiles = [(i * SCW, SCW) for i in range(nsc - 1)]
        tiles += [(last0, 256), (last0 + 256, 256)]

        for s0, scw in tiles:
            is_last = s0 + scw == seq
            ew = nc.gpsimd
            cpc = scw // CHUNK
            gc0 = s0 // CHUNK

            # both fp8 x streams for this seq-tile; the first tile is split
            # per-k so the PE can start after the first k-slice lands
            xh_t = xp.tile([128, NK, SCW], f8, tag="xh")
            xl_t = xp.tile([128, NK, SCW], f8, tag="xl")
            xh = xh_t[:, :, 0:scw]
            xl = xl_t[:, :, 0:scw]
            if s0 == 0:
                # startup: 2-k pieces first so pass-1 can begin ~3us in, then
                # quarters, split across the SP and ACT queues in the k-major
                # consumption order of the first tile's matmuls
                nc.sync.dma_start(
                    xh[:, 0:2, :],
                    xhiT[0:256, s0 : s0 + scw].rearrange("(k p) s -> p k s", p=128),
                )
                nc.sync.dma_start(
                    whi[:, 0:2, 512:640],
                    whiT[0:256, 512:640].rearrange("(k p) m -> p k m", p=128),
                )
                nc.sync.dma_start(
                    whi[:, 0:2, 0:384],
                    whiT[0:256, 0:384].rearrange("(k p) m -> p k m", p=128),
                )
                for k0_, k1_ in ((2, 4), (4, 8), (8, 12), (12, 16)):
                    nc.sync.dma_start(
                        xh[:, k0_:k1_, :],
                        xhiT[k0_ * 128 : k1_ * 128, s0 : s0 + scw].rearrange(
                            "(k p) s -> p k s", p=128
                        ),
                    )
                for k0_, k1_ in ((2, 4), (4, 8), (8, 12), (12, 16)):
                    nc.scalar.dma_start(
                        whi[:, k0_:k1_, :],
                        whiT[k0_ * 128 : k1_ * 128, :].rearrange(
                            "(k p) m -> p k m", p=128
                        ),
                    )
                for k0_ in range(0, NK, 4):
                    nc.scalar.dma_start(
                        xl[:, k0_ : k0_ + 4, :],
                        xloT[k0_ * 128 : (k0_ + 4) * 128, s0 : s0 + scw].rearrange(
                            "(k p) s -> p k s", p=128
                        ),
                    )
                nc.sync.dma_start(
                    wlo[:, :, :], wloT[:, :].rearrange("(k p) m -> p k m", p=128)
                )
            else:
                nc.sync.dma_start(
                    xh,
                    xhiT[:, s0 : s0 + scw].rearrange("(k p) s -> p k s", p=128),
                )
                nc.sync.dma_start(
                    xl,
                    xloT[:, s0 : s0 + scw].rearrange("(k p) s -> p k s", p=128),
                )
            ps = {}
            nh = (scw + 255) // 256  # N-halves of <=256 cols
            # B-tiles (t3/t4) first so the B drain overlaps the A matmuls.
            # On the final piece: t3 first (so the iB realign DMA's ~2.2us
            # latency hides under the matmuls), t4 (ivB's input) last.
            pool_of = {3: pp, 4: pv, 0: pv0, 1: pp, 2: pv}
            tseq = (3, 0, 1, 2, 4) if is_last else (3, 4, 0, 1, 2)
            torder = tuple((t, pool_of[t]) for t in tseq)
            if s0 == 0:
                # k-major emission for the first tile: every matmul consumes
                # the most recently landed DMA slice, so the PE ramps with
                # the startup stream instead of stalling on pass-2/3 inputs.
                # The two 256-col halves run as separate phases because one
                # psum tile cannot hold two pending accumulation groups.
                for t, pool in torder:
                    p = pool.tile([128, SCW], f32, tag=f"ps{t}")
                    ps[t] = p
                passes = [(xh, whi), (xl, whi), (xh, wlo)]
                for hh in range(nh):
                    n0 = hh * 256
                    n1 = min(n0 + 256, scw)
                    for pi, (xop, wop) in enumerate(passes):
                        for ks in range(NKS):
                            for t, _pool in torder:
                                m0 = t * 128
                                nc.tensor.matmul(
                                    ps[t][:, n0:n1],
                                    wop[:, 2 * ks : 2 * ks + 2, m0 : m0 + 128],
                                    xop[:, 2 * ks : 2 * ks + 2, n0:n1],
                                    start=(pi == 0 and ks == 0),
                                    stop=(pi == 2 and ks == NKS - 1),
                                    perf_mode=PM.DoubleRow,
                                )
                for t, _pool in torder:
                    ps[t] = ps[t][:, 0:scw]
            else:
                for t, pool in torder:
                    p = pool.tile([128, SCW], f32, tag=f"ps{t}")
                    m0 = t * 128
                    for hh in range(nh):
                        n0 = hh * 256
                        n1 = min(n0 + 256, scw)
                        seqs = []
                        for xop, wop in ((xh, whi), (xl, whi), (xh, wlo)):
                            for ks in range(NKS):
                                seqs.append((xop, wop, ks))
                        for j, (xop, wop, ks) in enumerate(seqs):
                            nc.tensor.matmul(
                                p[:, n0:n1],
                                wop[:, 2 * ks : 2 * ks + 2, m0 : m0 + 128],
                                xop[:, 2 * ks : 2 * ks + 2, n0:n1],
                                start=(j == 0),
                                stop=(j == len(seqs) - 1),
                                perf_mode=PM.DoubleRow,
                            )
                    ps[t] = p[:, 0:scw]

            def wkt(tag, pg, dt=f32, width=None):
                if width is not None:  # small chain tiles
                    t_ = wk.tile([pg, 8], dt, tag=tag)
                    return t_[:, 0:width]
                t_ = wk.tile([pg, SCW], dt, tag=tag)
                return t_[:, 0:scw]

            # B-group first throughout: its tiles (t3, t4) finish matmul
            # first, so its whole drain chain overlaps the A matmuls and the
            # end-of-kernel tail is only A's short chain. On the final piece
            # the whole ordering flips to A-major (A tiles drain first, B is
            # the short tail).
            qA = wkt("qA", 128)
            qB = wkt("qB", 64)
            prep_out = {}

            def emit_prep_B():
                ab = wkt("ab", 128)  # [aB; iB] + [db1; 0]
                nc.scalar.activation(
                    ab, ps[3], AF.Sigmoid, bias=db1_t[:], scale=PS_SCALE
                )
                iB = wkt("iB", 64)
                nc.scalar.dma_start(iB, ab[64:128, :])  # realign iB to rows 0..63
                prep_out["B"] = (ab[0:64, :], iB, ps[4][0:64, :])

            def emit_prep_A():
                aA = wkt("aA", 128)
                nc.scalar.activation(
                    aA, ps[0], AF.Sigmoid, bias=db0_t[:], scale=PS_SCALE
                )
                iA = wkt("iA", 128)
                nc.scalar.activation(iA, ps[1], AF.Sigmoid, scale=PS_SCALE)
                prep_out["A"] = (aA, iA, ps[2])

            def emit_q(name):
                pg, src, qb, qt = {
                    "B": (64, ps[3], qbB, qB),
                    "A": (128, ps[0], qbA, qA),
                }[name]
                sgs = []
                for idx, (_c, al, _b) in enumerate(QTERMS):
                    s = wkt(f"qs{name}{idx}", pg)
                    nc.scalar.activation(
                        s, src[0:pg, :], AF.Sigmoid,
                        bias=qb[idx][0:pg, :], scale=al * PS_SCALE,
                    )
                    sgs.append(s)
                # scalar_tensor_tensor is DVE-only on HW; build q on Pool
                # with in-place tensor_scalar + tensor_tensor ops.
                # Output coefficients carry PS_SCALE so q' = q/2048 and the
                # scan input w = q'*R*(i*psv) is true-scale.
                ew.tensor_scalar(
                    sgs[0], sgs[0], QTERMS[0][0] * PS_SCALE, QC0 * PS_SCALE,
                    ALU.mult, ALU.add,
                )
                ew.tensor_scalar(
                    sgs[1], sgs[1], QTERMS[1][0] * PS_SCALE, None, ALU.mult
                )
                ew.tensor_scalar(
                    sgs[2], sgs[2], QTERMS[2][0] * PS_SCALE, None, ALU.mult
                )
                ew.tensor_add(sgs[0], sgs[0], sgs[1])
                ew.tensor_add(qt, sgs[0], sgs[2])

            if is_last:
                emit_prep_B()
                emit_q("B")
                emit_prep_A()
                emit_q("A")
            else:
                emit_prep_B()
                emit_q("B")
                emit_q("A")
                emit_prep_A()

            gparams = {
                "B": ("B", 64, qB, CDb, CWb, INb),
                "A": ("A", 128, qA, CDa, CWa, INa),
            }
            gorder = ("A", "B") if is_last else ("B", "A")
            for gname in gorder:
                name, pg, q_t, CD, CW, IN = gparams[gname]
                a_ap, i_t, v_ps = prep_out[gname]
                # intra-chunk: D = cumprod(a) with chunk resets via
                # one masked scan: D = (a*m)*D_prev + a*(1-m)
                am = wkt(f"am{name}", pg)
                ew.tensor_mul(am, a_ap, mask[0:pg, 0:scw])
                az = wkt(f"az{name}", pg)
                ew.tensor_tensor(az, a_ap, am, ALU.subtract)
                D = wkt(f"D{name}", pg)
                nc.vector.tensor_tensor_scan(
                    D, am, az, 1.0, ALU.mult, ALU.add
                )
                Dc = wkt(f"Dc{name}", pg)
                ew.tensor_scalar_max(Dc, D, EPS_LOG)
                R = wkt(f"R{name}", pg)
                nc.vector.reciprocal_approx_fast(R, Dc)
                # r2 = q*R is ready before iv (iv needs the last matmul in
                # the drain tile), so w = r2*iv is one op after iv lands
                ew.tensor_mul(R, q_t, R)
                iv_t = wkt(f"iv{name}", pg)
                nc.vector.tensor_mul(iv_t, i_t, v_ps)
                w_ = iv_t
                ew.tensor_mul(w_, R, iv_t)
                W = wkt(f"W{name}", pg)
                nc.vector.tensor_tensor_scan(
                    W, mask[0:pg, 0:scw], w_, 0.0, ALU.mult, ALU.add
                )

                # incremental cross-chunk scan on this tile's boundaries
                g = slice(gc0, gc0 + cpc)
                bd = D[:, CHUNK - 1 :: CHUNK]
                bW = W[:, CHUNK - 1 :: CHUNK]
                bdc = wkt(f"bdc{name}", pg, width=cpc)
                ew.tensor_scalar_max(bdc, bd, EPS_LOG)
                itb = wkt(f"itb{name}", pg, width=cpc)
                ew.tensor_mul(itb, bd, bW)
                cd_init = 1.0 if s0 == 0 else CD[:, gc0 - 1 : gc0]
                nc.vector.tensor_tensor_scan(
                    CD[:, g], bdc, zeros[0:pg, 0:cpc], cd_init,
                    ALU.mult, ALU.add,
                )
                CDc = wkt(f"CDc{name}", pg, width=cpc)
                nc.vector.tensor_scalar_max(CDc, CD[:, g], EPS_LOG)
                CDr = wkt(f"CDr{name}", pg, width=cpc)
                nc.vector.reciprocal_approx_fast(CDr, CDc)
                tms = wkt(f"tms{name}", pg, width=cpc)
                nc.vector.tensor_mul(tms, itb, CDr)
                cw_init = 0.0 if s0 == 0 else CW[:, gc0 - 1 : gc0]
                nc.vector.tensor_tensor_scan(
                    CW[:, g], ones[0:pg, 0:cpc], tms, cw_init,
                    ALU.mult, ALU.add,
                )
                nc.vector.tensor_mul(IN[:, g], CD[:, g], CW[:, g])

                # combine: state = (W + inc) * D, written bf16
                ob = wkt(f"ob{name}", pg, dt=bf16)
                for c in range(cpc):
                    gc = gc0 + c
                    cs = slice(c * CHUNK, (c + 1) * CHUNK)
                    inc = (
                        zeros[0:pg, 0:1] if gc == 0
                        else IN[:, gc - 1 : gc]
                    )
                    nc.vector.scalar_tensor_tensor(
                        ob[:, cs], W[:, cs], inc, D[:, cs],
                        ALU.add, ALU.mult,
                    )
                orow = 0 if name == "A" else 128
                nc.scalar.dma_start(
                    out[orow : orow + pg, s0 : s0 + scw], ob
                )


def _build(seq):
    if seq in _built:
        return _built[seq]
    nc = bacc.Bacc(
        "TRN2", target_bir_lowering=False, debug=False, num_devices=NCORE
    )
    xhiT = nc.dram_tensor("xhiT", [D_MODEL, seq], f8, kind="ExternalInput").ap()
    xloT = nc.dram_tensor("xloT", [D_MODEL, seq], f8, kind="ExternalInput").ap()
    whiT = nc.dram_tensor("whiT", [D_MODEL, NT * 128], f8, kind="ExternalInput").ap()
    wloT = nc.dram_tensor("wloT", [D_MODEL, NT * 128], f8, kind="ExternalInput").ap()
    db0 = nc.dram_tensor("db0", [128, 1], f32, kind="ExternalInput").ap()
    db1 = nc.dram_tensor("db1", [128, 1], f32, kind="ExternalInput").ap()
    out = nc.dram_tensor("out", [CH, seq], bf16, kind="ExternalOutput").ap()
    with tile.TileContext(nc) as tc:
        _emit(tc, nc, xhiT, xloT, whiT, wloT, db0, db1, out, seq)
    nc.compile()
    _built[seq] = nc
    return nc


def _pack_w(W, h):
    """Pack this half's W rows into 640 rows of 5 M-tiles.

    t0 = a[0:128], t1 = i[0:128], t2 = v[0:128],
    t3 = [a[128:192]; i[128:192]], t4 = [v[128:192]; zeros]."""
    c0 = h * CH
    z = np.zeros((64, W.shape[1]), np.float32)
    return np.concatenate(
        [
            W[c0 : c0 + 128],
            W[D_REC + c0 : D_REC + c0 + 128],
            W[2 * D_REC + c0 : 2 * D_REC + c0 + 128],
            W[c0 + 128 : c0 + 192],
            W[D_REC + c0 + 128 : D_REC + c0 + 192],
            W[2 * D_REC + c0 + 128 : 2 * D_REC + c0 + 192],
            z,
        ],
        axis=0,
    )


def _in_maps(x, W, db):
    maps = []
    xhi_c, xlo_c = {}, {}
    for core in range(NCORE):
        b, hh = core // 2, core % 2
        if b not in xhi_c:
            xb = x[b]  # [seq, D_MODEL] f32
            xhi = xb.astype(E4).astype(np.float32)
            xhi_c[b] = np.ascontiguousarray((32.0 * xhi).T).astype(E4)
            xlo_c[b] = np.ascontiguousarray((32.0 * (xb - xhi)).T).astype(E4)
        w64 = 64.0 * _pack_w(W, hh)
        whi = w64.astype(E4).astype(np.float32)
        wlo = w64 - whi
        c0 = hh * CH
        db0v = np.ascontiguousarray(db[c0 : c0 + 128].reshape(128, 1))
        db1v = np.ascontiguousarray(
            np.concatenate([db[c0 + 128 : c0 + 192], np.zeros(64, np.float32)]).reshape(
                128, 1
            )
        )
        maps.append(
            {
                "xhiT": xhi_c[b],
                "xloT": xlo_c[b],
                "whiT": np.ascontiguousarray(whi.T).astype(E4),
                "wloT": np.ascontiguousarray(wlo.T).astype(E4),
                "db0": db0v,
                "db1": db1v,
            }
        )
    return maps


def kernel(x, W, decay_bias, _trace=False):
    x = np.asarray(x, np.float32)
    W = np.asarray(W, np.float32)
    db = np.asarray(decay_bias, np.float32)
    B, S, _ = x.shape
    nc = _build(S)
    res = run_bass_kernel_spmd(nc, _in_maps(x, W, db), list(range(NCORE)), trace=_trace)
    outf = np.empty((B, S, D_REC), np.float32)
    for core in range(NCORE):
        b, hh = core // 2, core % 2
        outf[b, :, hh * CH : (hh + 1) * CH] = (
            np.asarray(res.results[core]["out"]).astype(np.float32).T
        )
    if _trace:
        return outf, res
    return outf


# revision 30
# speedup vs baseline: 1.0119x; 1.0044x over previous
"""Griffin recurrence Trainium2 kernel, v3 (fp8 DoubleRow matmul).

Sharding: 8 cores = 4 batches x 2 channel-halves (192 channels each).
Matmul runs in fp8e4 with MatmulPerfMode.DoubleRow (256-deep contraction
per instruction, 0.5 cycles per output column = 4x the bf16 rate) using a
3-pass hi/lo split that keeps every pass at one common PSUM scale:

    xhi32 = 32*fp8(x)            (exact exponent shift)
    xlo32 = fp8(32*(x - fp8(x)))
    Whi   = fp8(64*W)
    Wlo   = fp8(64*W - Whi)      (direct, subnormals fine)
    psum  = xhi32@Whi + xlo32@Whi + xhi32@Wlo = 2048*(x@W) + O(err)

The 1/2048 descale is folded into the ACT scale arguments and into the
fitted q-sigmoid output coefficients, so no extra elementwise ops.
The chunked scan is identical to v2: per-chunk cumulative decay via a
multiplicative tensor_tensor_scan, clipped division u*recip(max(D,1e-10)),
cumsum scan, and an incremental cross-chunk scan chained through AP scan
initials; sqrt(1-a^2) is a fitted sum of sigmoids so the ACT engine never
swaps activation tables. Elementwise work is split across Pool (gpsimd)
and DVE; output is written bf16.

HW-legality notes (birverifier/codegen): GPSIMD cannot access PSUM;
scalar_tensor_tensor is DVE-only; TT divide is not a valid HW ALU op.
"""

import sys

sys.path.insert(0, "/opt/trn_rl_repo")

from contextlib import ExitStack

import numpy as np
import ml_dtypes

from concourse import bacc, mybir, tile
from concourse.bass_utils import run_bass_kernel_spmd

f32 = mybir.dt.float32
bf16 = mybir.dt.bfloat16
f8 = mybir.dt.float8e4
AF = mybir.ActivationFunctionType
ALU = mybir.AluOpType
PM = mybir.MatmulPerfMode

D_MODEL = 2048
D_REC = 384
CHUNK = 64
NCORE = 8
CH = 192           # channels per core
SCW = 512          # seq-tile width in the steady state
NK = D_MODEL // 128  # 16 k-subtiles (8 DoubleRow supertiles)
NKS = NK // 2      # 8 k-supertiles of 256
NT = 5             # M-tiles (640 = 5*128 packed W rows)
EPS_LOG = 1e-10
PS_SCALE = 1.0 / 2048.0  # psum holds 2048*z

E4 = ml_dtypes.float8_e4m3

# q(p) = sqrt(1 - sigmoid(p)^2) ~= C0 + sum_i Ci*sigmoid(Ai*p + Bi),
# max abs err 8.8e-4 over p in [-14, 14]. All terms live in the sigmoid
# ACT table, so the kernel never swaps activation tables. The output
# coefficients are multiplied by PS_SCALE at emit time to fold in the
# fp8 psum descale (q' = q/2048 so u = q'*(i*psv) is true-scale).
QC0 = 0.697865
QTERMS = (
    (0.302546, -0.537589, 1.749442),
    (-3.0, 0.831839, -0.394654),
    (2.303064, 0.830397, -0.058968),
)

_built = {}


def _emit(tc, nc, xhiT, xloT, whiT, wloT, db0, db1, out, seq):
    nsc = seq // SCW
    nch = seq // CHUNK        # 64 chunks total

    with ExitStack() as ctx:
        const = ctx.enter_context(tc.tile_pool(name="const", bufs=1))
        sm = ctx.enter_context(tc.tile_pool(name="sm", bufs=1))
        xp = ctx.enter_context(tc.tile_pool(name="xp", bufs=3))
        pp = ctx.enter_context(tc.tile_pool(name="pp", bufs=1, space="PSUM"))
        pv = ctx.enter_context(tc.tile_pool(name="pv", bufs=2, space="PSUM"))
        pv0 = ctx.enter_context(tc.tile_pool(name="pv0", bufs=2, space="PSUM"))
        wk = ctx.enter_context(tc.tile_pool(name="wk", bufs=2))

        # constants. x stream owns the SP queue; W/db go on the ACT queue
        # (k-ascending W so the PE can start immediately).
        zeros = const.tile([128, CHUNK], f32, tag="zeros")
        nc.vector.memset(zeros[:], 0.0)
        ones = const.tile([128, CHUNK], f32, tag="ones")
        nc.vector.memset(ones[:], 1.0)
        # W-scan mask: ones with zeros at chunk starts (segmented cumsum)
        mask = const.tile([128, SCW], f32, tag="mask")
        nc.vector.memset(mask[:], 1.0)
        for c0_ in range(0, SCW, CHUNK):
            nc.vector.memset(mask[:, c0_ : c0_ + 1], 0.0)
        whi = const.tile([128, NK, NT * 128], f8, tag="whi")
        wlo = const.tile([128, NK, NT * 128], f8, tag="wlo")
        # t3 column of k0 first, on the SP queue: the ACT queue opens with
        # a hoisted LoadActFuncSet (1.3us) that would gate the first
        # matmul's weights. The rest of the W/x startup stream is
        # interleaved across the SP and ACT queues (see the s0 == 0 branch
        # below); wlo arrives as one bulk DMA once the hi stream is in.
        nc.sync.dma_start(
            whi[:, 0:2, 384:512],
            whiT[0:256, 384:512].rearrange("(k p) m -> p k m", p=128),
        )
        db0_t = const.tile([128, 1], f32, tag="db0")
        nc.scalar.dma_start(db0_t[:], db0[:])
        db1_t = const.tile([128, 1], f32, tag="db1")
        nc.scalar.dma_start(db1_t[:], db1[:])
        # bias columns for the q sigmoid terms: Ai*db + Bi
        qbA, qbB = [], []
        for idx, (_c, al, be) in enumerate(QTERMS):
            ta = const.tile([128, 1], f32, tag=f"qbA{idx}")
            nc.vector.tensor_scalar(ta[:], db0_t[:], al, be, ALU.mult, ALU.add)
            qbA.append(ta)
            tb = const.tile([64, 1], f32, tag=f"qbB{idx}")
            nc.vector.tensor_scalar(tb[:], db1_t[0:64, :], al, be, ALU.mult, ALU.add)
            qbB.append(tb)

        # cross-chunk chains (persistent, written cpc columns per seq-tile)
        CDa = sm.tile([128, nch], f32, tag="CDa")
        CWa = sm.tile([128, nch], f32, tag="CWa")
        INa = sm.tile([128, nch], f32, tag="INa")
        CDb = sm.tile([64, nch], f32, tag="CDb")
        CWb = sm.tile([64, nch], f32, tag="CWb")
        INb = sm.tile([64, nch], f32, tag="INb")

        # steady-state 512-wide tiles; the last 512 is split into two 256s so
        # the post-matmul drain chain after the final matmul is short.
        last0 = (nsc - 1) * SCW
        tiles = [(i * SCW, SCW) for i in range(nsc - 1)]
        tiles += [(last0, 256), (last0 + 256, 256)]

        for s0, scw in tiles:
            is_last = s0 + scw == seq
            ew = nc.gpsimd
            cpc = scw // CHUNK
            gc0 = s0 // CHUNK

            # both fp8 x streams for this seq-tile; the first tile is split
            # per-k so the PE can start after the first k-slice lands
            xh_t = xp.tile([128, NK, SCW], f8, tag="xh")
            xl_t = xp.tile([128, NK, SCW], f8, tag="xl")
            xh = xh_t[:, :, 0:scw]
            xl = xl_t[:, :, 0:scw]
            if s0 == 0:
                # startup: 2-k pieces first so pass-1 can begin ~3us in, then
                # quarters, split across the SP and ACT queues in the k-major
                # consumption order of the first tile's matmuls
                nc.sync.dma_start(
                    xh[:, 0:2, :],
                    xhiT[0:256, s0 : s0 + scw].rearrange("(k p) s -> p k s", p=128),
                )
                nc.sync.dma_start(
                    whi[:, 0:2, 512:640],
                    whiT[0:256, 512:640].rearrange("(k p) m -> p k m", p=128),
                )
                nc.sync.dma_start(
                    whi[:, 0:2, 0:384],
                    whiT[0:256, 0:384].rearrange("(k p) m -> p k m", p=128),
                )
                for k0_, k1_ in ((2, 4), (4, 8), (8, 12), (12, 16)):
                    nc.sync.dma_start(
                        xh[:, k0_:k1_, :],
                        xhiT[k0_ * 128 : k1_ * 128, s0 : s0 + scw].rearrange(
                            "(k p) s -> p k s", p=128
                        ),
                    )
                for k0_, k1_ in ((2, 4), (4, 8), (8, 12), (12, 16)):
                    nc.scalar.dma_start(
                        whi[:, k0_:k1_, :],
                        whiT[k0_ * 128 : k1_ * 128, :].rearrange(
                            "(k p) m -> p k m", p=128
                        ),
                    )
                for k0_ in range(0, NK, 4):
                    nc.scalar.dma_start(
                        xl[:, k0_ : k0_ + 4, :],
                        xloT[k0_ * 128 : (k0_ + 4) * 128, s0 : s0 + scw].rearrange(
                            "(k p) s -> p k s", p=128
                        ),
                    )
                nc.sync.dma_start(
                    wlo[:, :, :], wloT[:, :].rearrange("(k p) m -> p k m", p=128)
                )
            else:
                nc.sync.dma_start(
                    xh,
                    xhiT[:, s0 : s0 + scw].rearrange("(k p) s -> p k s", p=128),
                )
                nc.sync.dma_start(
                    xl,
                    xloT[:, s0 : s0 + scw].rearrange("(k p) s -> p k s", p=128),
                )
            ps = {}
            nh = (scw + 255) // 256  # N-halves of <=256 cols
            # B-tiles (t3/t4) first so the B drain overlaps the A matmuls
            # and the iB realign DMA's ~2.2us latency hides under them.
            pool_of = {3: pp, 4: pv, 0: pv0, 1: pp, 2: pv}
            tseq = (3, 4, 0, 1, 2)
            torder = tuple((t, pool_of[t]) for t in tseq)
            if s0 == 0:
                # k-major emission for the first tile: every matmul consumes
                # the most recently landed DMA slice, so the PE ramps with
                # the startup stream instead of stalling on pass-2/3 inputs.
                # The two 256-col halves run as separate phases because one
                # psum tile cannot hold two pending accumulation groups.
                for t, pool in torder:
                    p = pool.tile([128, SCW], f32, tag=f"ps{t}")
                    ps[t] = p
                passes = [(xh, whi), (xl, whi), (xh, wlo)]
                for hh in range(nh):
                    n0 = hh * 256
                    n1 = min(n0 + 256, scw)
                    for pi, (xop, wop) in enumerate(passes):
                        for ks in range(NKS):
                            for t, _pool in torder:
                                m0 = t * 128
                                nc.tensor.matmul(
                                    ps[t][:, n0:n1],
                                    wop[:, 2 * ks : 2 * ks + 2, m0 : m0 + 128],
                                    xop[:, 2 * ks : 2 * ks + 2, n0:n1],
                                    start=(pi == 0 and ks == 0),
                                    stop=(pi == 2 and ks == NKS - 1),
                                    perf_mode=PM.DoubleRow,
                                )
                for t, _pool in torder:
                    ps[t] = ps[t][:, 0:scw]
            else:
                for t, pool in torder:
                    p = pool.tile([128, SCW], f32, tag=f"ps{t}")
                    m0 = t * 128
                    for hh in range(nh):
                        n0 = hh * 256
                        n1 = min(n0 + 256, scw)
                        seqs = []
                        for xop, wop in ((xh, whi), (xl, whi), (xh, wlo)):
                            for ks in range(NKS):
                                seqs.append((xop, wop, ks))
                        for j, (xop, wop, ks) in enumerate(seqs):
                            nc.tensor.matmul(
                                p[:, n0:n1],
                                wop[:, 2 * ks : 2 * ks + 2, m0 : m0 + 128],
                                xop[:, 2 * ks : 2 * ks + 2, n0:n1],
                                start=(j == 0),
                                stop=(j == len(seqs) - 1),
                                perf_mode=PM.DoubleRow,
                            )
                    ps[t] = p[:, 0:scw]

            def wkt(tag, pg, dt=f32, width=None):
                if width is not None:  # small chain tiles
                    t_ = wk.tile([pg, 8], dt, tag=tag)
                    return t_[:, 0:width]
                t_ = wk.tile([pg, SCW], dt, tag=tag)
                return t_[:, 0:scw]

            # B-group first throughout: its tiles (t3, t4) finish matmul
            # first, so its whole drain chain overlaps the A matmuls and the
            # end-of-kernel tail is only A's short chain. On the final piece
            # the whole ordering flips to A-major (A tiles drain first, B is
            # the short tail).
            qA = wkt("qA", 128)
            qB = wkt("qB", 64)
            prep_out = {}

            def emit_prep_B():
                ab = wkt("ab", 128)  # [aB; iB] + [db1; 0]
                nc.scalar.activation(
                    ab, ps[3], AF.Sigmoid, bias=db1_t[:], scale=PS_SCALE
                )
                iB = wkt("iB", 64)
                nc.scalar.dma_start(iB, ab[64:128, :])  # realign iB to rows 0..63
                prep_out["B"] = (ab[0:64, :], iB, ps[4][0:64, :])

            def emit_prep_A():
                aA = wkt("aA", 128)
                nc.scalar.activation(
                    aA, ps[0], AF.Sigmoid, bias=db0_t[:], scale=PS_SCALE
                )
                iA = wkt("iA", 128)
                nc.scalar.activation(iA, ps[1], AF.Sigmoid, scale=PS_SCALE)
                prep_out["A"] = (aA, iA, ps[2])

            def emit_q(name):
                pg, src, qb, qt = {
                    "B": (64, ps[3], qbB, qB),
                    "A": (128, ps[0], qbA, qA),
                }[name]
                sgs = []
                for idx, (_c, al, _b) in enumerate(QTERMS):
                    s = wkt(f"qs{name}{idx}", pg)
                    nc.scalar.activation(
                        s, src[0:pg, :], AF.Sigmoid,
                        bias=qb[idx][0:pg, :], scale=al * PS_SCALE,
                    )
                    sgs.append(s)
                # scalar_tensor_tensor is DVE-only on HW; build q on Pool
                # with in-place tensor_scalar + tensor_tensor ops.
                # Output coefficients carry PS_SCALE so q' = q/2048 and the
                # scan input w = q'*R*(i*psv) is true-scale.
                ew.tensor_scalar(
                    sgs[0], sgs[0], QTERMS[0][0] * PS_SCALE, QC0 * PS_SCALE,
                    ALU.mult, ALU.add,
                )
                ew.tensor_scalar(
                    sgs[1], sgs[1], QTERMS[1][0] * PS_SCALE, None, ALU.mult
                )
                ew.tensor_scalar(
                    sgs[2], sgs[2], QTERMS[2][0] * PS_SCALE, None, ALU.mult
                )
                ew.tensor_add(sgs[0], sgs[0], sgs[1])
                ew.tensor_add(qt, sgs[0], sgs[2])

            if is_last:
                # Tail-critical ACT order: ab then aA immediately (the long
                # A decay-path runs under the remaining matmuls), q sigmoids
                # next, and iA (whose ps1 input lands late) last so it never
                # blocks the queue.
                emit_prep_B()
                aA = wkt("aA", 128)
                nc.scalar.activation(
                    aA, ps[0], AF.Sigmoid, bias=db0_t[:], scale=PS_SCALE
                )
                emit_q("A")
                emit_q("B")
                iA = wkt("iA", 128)
                nc.scalar.activation(iA, ps[1], AF.Sigmoid, scale=PS_SCALE)
                prep_out["A"] = (aA, iA, ps[2])
            else:
                emit_prep_B()
                emit_q("B")
                emit_q("A")
                emit_prep_A()

            gparams = {
                "B": ("B", 64, qB, CDb, CWb, INb),
                "A": ("A", 128, qA, CDa, CWa, INa),
            }
            gorder = ("A", "B") if is_last else ("B", "A")
            for gname in gorder:
                name, pg, q_t, CD, CW, IN = gparams[gname]
                a_ap, i_t, v_ps = prep_out[gname]
                # intra-chunk: D = cumprod(a) with chunk resets via
                # one masked scan: D = (a*m)*D_prev + a*(1-m)
                am = wkt(f"am{name}", pg)
                ew.tensor_mul(am, a_ap, mask[0:pg, 0:scw])
                az = wkt(f"az{name}", pg)
                ew.tensor_tensor(az, a_ap, am, ALU.subtract)
                D = wkt(f"D{name}", pg)
                nc.vector.tensor_tensor_scan(
                    D, am, az, 1.0, ALU.mult, ALU.add
                )
                Dc = wkt(f"Dc{name}", pg)
                ew.tensor_scalar_max(Dc, D, EPS_LOG)
                R = wkt(f"R{name}", pg)
                nc.vector.reciprocal_approx_fast(R, Dc)
                # r2 = q*R is ready before iv (iv needs the last matmul in
                # the drain tile), so w = r2*iv is one op after iv lands
                ew.tensor_mul(R, q_t, R)
                iv_t = wkt(f"iv{name}", pg)
                nc.vector.tensor_mul(iv_t, i_t, v_ps)
                w_ = iv_t
                ew.tensor_mul(w_, R, iv_t)
                W = wkt(f"W{name}", pg)
                nc.vector.tensor_tensor_scan(
                    W, mask[0:pg, 0:scw], w_, 0.0, ALU.mult, ALU.add
                )

                # incremental cross-chunk scan on this tile's boundaries
                g = slice(gc0, gc0 + cpc)
                bd = D[:, CHUNK - 1 :: CHUNK]
                bW = W[:, CHUNK - 1 :: CHUNK]
                bdc = wkt(f"bdc{name}", pg, width=cpc)
                ew.tensor_scalar_max(bdc, bd, EPS_LOG)
                itb = wkt(f"itb{name}", pg, width=cpc)
                ew.tensor_mul(itb, bd, bW)
                cd_init = 1.0 if s0 == 0 else CD[:, gc0 - 1 : gc0]
                nc.vector.tensor_tensor_scan(
                    CD[:, g], bdc, zeros[0:pg, 0:cpc], cd_init,
                    ALU.mult, ALU.add,
                )
                CDc = wkt(f"CDc{name}", pg, width=cpc)
                nc.vector.tensor_scalar_max(CDc, CD[:, g], EPS_LOG)
                CDr = wkt(f"CDr{name}", pg, width=cpc)
                nc.vector.reciprocal_approx_fast(CDr, CDc)
                tms = wkt(f"tms{name}", pg, width=cpc)
                nc.vector.tensor_mul(tms, itb, CDr)
                cw_init = 0.0 if s0 == 0 else CW[:, gc0 - 1 : gc0]
                nc.vector.tensor_tensor_scan(
                    CW[:, g], ones[0:pg, 0:cpc], tms, cw_init,
                    ALU.mult, ALU.add,
                )
                nc.vector.tensor_mul(IN[:, g], CD[:, g], CW[:, g])

                # combine: state = (W + inc) * D, written bf16
                ob = wkt(f"ob{name}", pg, dt=bf16)
                for c in range(cpc):
                    gc = gc0 + c
                    cs = slice(c * CHUNK, (c + 1) * CHUNK)
                    inc = (
                        zeros[0:pg, 0:1] if gc == 0
                        else IN[:, gc - 1 : gc]
                    )
                    nc.vector.scalar_tensor_tensor(
                        ob[:, cs], W[:, cs], inc, D[:, cs],
                        ALU.add, ALU.mult,
                    )
                orow = 0 if name == "A" else 128
                nc.scalar.dma_start(
                    out[orow : orow + pg, s0 : s0 + scw], ob
                )


def _build(seq):
    if seq in _built:
        return _built[seq]
    nc = bacc.Bacc(
        "TRN2", target_bir_lowering=False, debug=False, num_devices=NCORE
    )
    xhiT = nc.dram_tensor("xhiT", [D_MODEL, seq], f8, kind="ExternalInput").ap()
    xloT = nc.dram_tensor("xloT", [D_MODEL, seq], f8, kind="ExternalInput").ap()
    whiT = nc.dram_tensor("whiT", [D_MODEL, NT * 128], f8, kind="ExternalInput").ap()
    wloT = nc.dram_tensor("wloT", [D_MODEL, NT * 128], f8, kind="ExternalInput").ap()
    db0 = nc.dram_tensor("db0", [128, 1], f32, kind="ExternalInput").ap()
    db1 = nc.dram_tensor("db1", [128, 1], f32, kind="ExternalInput").ap()
    out = nc.dram_tensor("out", [CH, seq], bf16, kind="ExternalOutput").ap()
    with tile.TileContext(nc) as tc:
        _emit(tc, nc, xhiT, xloT, whiT, wloT, db0, db1, out, seq)
    nc.compile()
    _built[seq] = nc
    return nc


def _pack_w(W, h):
    """Pack this half's W rows into 640 rows of 5 M-tiles.

    t0 = a[0:128], t1 = i[0:128], t2 = v[0:128],
    t3 = [a[128:192]; i[128:192]], t4 = [v[128:192]; zeros]."""
    c0 = h * CH
    z = np.zeros((64, W.shape[1]), np.float32)
    return np.concatenate(
        [
            W[c0 : c0 + 128],
            W[D_REC + c0 : D_REC + c0 + 128],
            W[2 * D_REC + c0 : 2 * D_REC + c0 + 128],
            W[c0 + 128 : c0 + 192],
            W[D_REC + c0 + 128 : D_REC + c0 + 192],
            W[2 * D_REC + c0 + 128 : 2 * D_REC + c0 + 192],
            z,
        ],
        axis=0,
    )


def _in_maps(x, W, db):
    maps = []
    xhi_c, xlo_c = {}, {}
    for core in range(NCORE):
        b, hh = core // 2, core % 2
        if b not in xhi_c:
            xb = x[b]  # [seq, D_MODEL] f32
            xhi = xb.astype(E4).astype(np.float32)
            xhi_c[b] = np.ascontiguousarray((32.0 * xhi).T).astype(E4)
            xlo_c[b] = np.ascontiguousarray((32.0 * (xb - xhi)).T).astype(E4)
        w64 = 64.0 * _pack_w(W, hh)
        whi = w64.astype(E4).astype(np.float32)
        wlo = w64 - whi
        c0 = hh * CH
        db0v = np.ascontiguousarray(db[c0 : c0 + 128].reshape(128, 1))
        db1v = np.ascontiguousarray(
            np.concatenate([db[c0 + 128 : c0 + 192], np.zeros(64, np.float32)]).reshape(
                128, 1
            )
        )
        maps.append(
            {
                "xhiT": xhi_c[b],
                "xloT": xlo_c[b],
                "whiT": np.ascontiguousarray(whi.T).astype(E4),
                "wloT": np.ascontiguousarray(wlo.T).astype(E4),
                "db0": db0v,
                "db1": db1v,
            }
        )
    return maps


def kernel(x, W, decay_bias, _trace=False):
    x = np.asarray(x, np.float32)
    W = np.asarray(W, np.float32)
    db = np.asarray(decay_bias, np.float32)
    B, S, _ = x.shape
    nc = _build(S)
    res = run_bass_kernel_spmd(nc, _in_maps(x, W, db), list(range(NCORE)), trace=_trace)
    outf = np.empty((B, S, D_REC), np.float32)
    for core in range(NCORE):
        b, hh = core // 2, core % 2
        outf[b, :, hh * CH : (hh + 1) * CH] = (
            np.asarray(res.results[core]["out"]).astype(np.float32).T
        )
    if _trace:
        return outf, res
    return outf
